# revision 37
# baseline (speedup 1.0000x reference)
"""Distributed Trainium2 kernel for the 4-layer single-head causal-attention
stack (returns mean attention weights over layers).

Sharding: sequence-parallel over the 2048 mentions. 16 row-tiles of 128;
core c owns tiles {c, 15-c} so causal-attention work is identical on every
core -> one uniform SPMD program. Per layer each core projects K,V for its
256 rows, all-gathers K,V across the 8 cores (one collective), projects Q
while the gather runs, then computes masked scores, softmax and W@V in
bf16 with f32 PSUM accumulation.

Key structural choices:
- The per-layer output projection is folded into the next layer's QKV
  weights on the host (W'_i = Wqkv_i @ Wo_{i-1}); x_i never materializes.
- Layer 3 only computes Q,K (its attention output is never consumed).
- K/V columns live in "rank-paired" order sigma = [0,15,1,14,...]: rank
  r's gathered block lands contiguously, so the K unpack is 8 line-rate
  DMAs and V unpack is 2 strided DMAs. Row-tile A (global tile c) only
  ever attends to global tiles 0..7 = the even sigma positions, read as a
  strided matmul operand, so the causal 25% FLOP saving survives the
  reordering. The host un-permutes the B rows' output columns.
- W^T for W@V comes from one batched DMA transpose of an interleaved
  [A0 B0 A1 B1 ...] buffer -> N=256 moving operands, no PE transposes.
- DMA count is kept low and split across both HWDGE rings (sync+scalar);
  PSUM->SBUF copies are spread over DVE and ACT.
"""

import numpy as np
import ml_dtypes

N, E, L, NCORES = 2048, 1024, 4, 8
EC = E // 128          # 8 contraction chunks of 128
MT = 256               # mention rows per core
SCALE = 1.0 / np.sqrt(np.float32(E))
KV_K_ELEMS = E * MT            # k block: [1024, 256] (feature-major)
KV_V_ELEMS = MT * E            # v block: [256, 1024] (row-major natural)
KV_ELEMS = KV_K_ELEMS + KV_V_ELEMS
NEG = -1e30

BF16 = ml_dtypes.bfloat16

# sigma: column-block position s holds global row-tile SIGMA_G[s]
SIGMA_G = [t for pair in ((u, 15 - u) for u in range(8)) for t in pair]
# inverse: global tile t lives at column-block position SIGMA_INV[t]
SIGMA_INV = [0] * 16
for _s, _g in enumerate(SIGMA_G):
    SIGMA_INV[_g] = _s

_RUNNER = None


def _build_nc():
    import concourse.mybir as mybir
    import concourse.tile as tile
    from concourse import bacc
    from contextlib import ExitStack

    f32 = mybir.dt.float32
    bf16 = mybir.dt.bfloat16
    f8 = mybir.dt.float8e4

    nc = bacc.Bacc("TRN2", target_bir_lowering=False, debug=False,
                   num_devices=NCORES)

    xt_p = nc.declare_dram_parameter("xt", [E, MT], bf16, isOutput=False)
    wqkvt_p = nc.declare_dram_parameter("wqkvt", [L * E, 3 * E], bf16, isOutput=False)
    bqkv_p = nc.declare_dram_parameter("bqkv", [L * 3 * E], f32, isOutput=False)
    maska_p = nc.declare_dram_parameter("maska", [128, 1024], bf16, isOutput=False)
    maskb_p = nc.declare_dram_parameter("maskb", [128, 2048], bf16, isOutput=False)
    out_p = nc.declare_dram_parameter("out", [MT, N], f32, isOutput=True)

    AOP = mybir.AluOpType
    AF = mybir.ActivationFunctionType

    with tile.TileContext(nc) as tc:
        with ExitStack() as stack:
            ep_ = lambda **kw: stack.enter_context(tc.tile_pool(**kw))
            dram = ep_(name="dram", bufs=2, space="DRAM")
            consts = ep_(name="consts", bufs=1)
            px = ep_(name="px", bufs=2)
            pq = ep_(name="pq", bufs=2)
            pktf = ep_(name="pktf", bufs=1)
            pvf = ep_(name="pvf", bufs=1)
            pscore = ep_(name="pscore", bufs=1)
            pw = ep_(name="pw", bufs=1)
            pacc = ep_(name="pacc", bufs=1)
            pwqk = ep_(name="pwqk", bufs=2)
            pwv = ep_(name="pwv", bufs=2)
            pstage = ep_(name="pstage", bufs=2)
            pbias = ep_(name="pbias", bufs=2)
            pstats = ep_(name="pstats", bufs=4)
            psmm = ep_(name="psmm", bufs=2, space="PSUM")
            pssc = ep_(name="pssc", bufs=3, space="PSUM")
            pssm = ep_(name="pssm", bufs=2, space="PSUM")
            psdz = ep_(name="psdz", bufs=1, space="PSUM")

            maska = consts.tile([128, 1024], bf16)
            nc.sync.dma_start(maska[:], maska_p[:, :])
            maskb = consts.tile([128, 2048], bf16)
            nc.sync.dma_start(maskb[:], maskb_p[:, :])
            zeros = consts.tile([128, 1024], f32)
            nc.vector.memset(zeros[:], 0.0)
            nc.sync.dma_start(out_p[0:128, 1024:2048], zeros[:])
            acc_a = pacc.tile([128, 1024], f32, tag="acca")
            nc.vector.memset(acc_a[:], 0.0)
            acc_b = pacc.tile([128, 2048], f32, tag="accb")
            nc.vector.memset(acc_b[:], 0.0)

            # zero scratch for PE keep-warm dummy matmuls (the HAM clock
            # gate halves the PE clock after ~4us idle; gathers idle the PE
            # for ~40-55us, so every post-gather matmul would run cold)
            dz = consts.tile([128, 512], bf16)
            nc.vector.memset(dz[:], 0.0)

            # w_ab slot 2t = tile-A's global-tile-t block (stays zero for
            # t>=8), slot 2t+1 = tile-B's global-tile-t block; reused (and
            # the zero slots memset) once across all layers
            w_ab = pw.tile([128, 4096], bf16, tag="wab")
            wab3 = w_ab[:].rearrange("p (s m) -> p s m", m=128)
            nc.gpsimd.memset(wab3[:, 16:32:2, :], 0.0)

            xt = px.tile([128, EC, MT], bf16, tag="xt")
            nc.sync.dma_start(
                xt[:], xt_p.ap().rearrange("(c p) m -> p c m", p=128))

            for li in range(L):
                last = li == L - 1
                wrow = li * E  # weight row offset for this layer

                bq = pbias.tile([128, 24], f32, tag="bq")
                nc.sync.dma_start(
                    bq[:],
                    bqkv_p.ap()[li * 3 * E:(li + 1) * 3 * E]
                    .rearrange("(c p) -> p c", p=128))

                kv_ks = dram.tile([KV_K_ELEMS], f8, tag="kvks")
                kv_kd = dram.tile([KV_K_ELEMS * NCORES], f8, tag="kvkd",
                                  addr_space="Shared")

                # ---- K projection (features 1024:2048 -> f_tiles 8..15) ----
                kstage = pstage.tile([128, 8, MT], f8, tag="kst")
                for kw in range(2):
                    wt = pwqk.tile([128, EC, 512], bf16, tag="wqk")
                    nc.sync.dma_start(
                        wt[:],
                        wqkvt_p.ap()[wrow:wrow + E,
                                     1024 + 512 * kw:1024 + 512 * (kw + 1)]
                        .rearrange("(c p) f -> p c f", p=128))
                    for fl in range(4):
                        ft = 8 + 4 * kw + fl
                        ps = psmm.tile([128, MT], f32, tag="mm")
                        for ec in range(EC):
                            nc.tensor.matmul(
                                ps[:], wt[:, ec, 128 * fl:128 * (fl + 1)],
                                xt[:, ec, :],
                                start=(ec == 0), stop=(ec == EC - 1))
                        nc.vector.tensor_scalar_add(kstage[:, ft - 8, :], ps[:],
                                                    bq[:, ft:ft + 1])
                nc.sync.dma_start(
                    kv_ks[:].rearrange("(c p m) -> p c m", p=128, m=MT),
                    kstage[:])
                nc.gpsimd.collective_compute(
                    "AllGather", AOP.bypass,
                    replica_groups=[list(range(NCORES))],
                    ins=[kv_ks[:].opt()],
                    outs=[kv_kd[:].opt()],
                )

                # ---- V projection (natural layout [m, e]) ----
                if not last:
                    kv_vs = dram.tile([KV_V_ELEMS], f8, tag="kvvs")
                    kv_vd = dram.tile([KV_V_ELEMS * NCORES], f8, tag="kvvd",
                                      addr_space="Shared")
                    vstage = pstage.tile([128, 2, E], f8, tag="vst")
                    for s in range(2):
                        wvt_w = pwv.tile([128, EC, 512], bf16, tag="wv")
                        nc.sync.dma_start(
                            wvt_w[:],
                            wqkvt_p.ap()[wrow:wrow + E,
                                         2048 + 512 * s:2048 + 512 * (s + 1)]
                            .rearrange("(c p) f -> p c f", p=128))
                        for mt in range(2):
                            ps = psmm.tile([128, 512], f32, tag="mm")
                            for ec in range(EC):
                                nc.tensor.matmul(
                                    ps[:], xt[:, ec, 128 * mt:128 * (mt + 1)],
                                    wvt_w[:, ec, :],
                                    start=(ec == 0), stop=(ec == EC - 1))
                            nc.scalar.copy(vstage[:, mt, 512 * s:512 * (s + 1)],
                                           ps[:])
                    nc.sync.dma_start(
                        kv_vs[:].rearrange("(t p e) -> p t e", t=2, p=128),
                        vstage[:])
                    nc.gpsimd.collective_compute(
                        "AllGather", AOP.bypass,
                        replica_groups=[list(range(NCORES))],
                        ins=[kv_vs[:].opt()],
                        outs=[kv_vd[:].opt()],
                    )

                # ---- Q projection (features 0:1024, pre-scaled weights) ----
                qt = pq.tile([128, EC, MT], bf16, tag="qt")
                for kw in range(2):
                    wt = pwqk.tile([128, EC, 512], bf16, tag="wqk")
                    nc.sync.dma_start(
                        wt[:],
                        wqkvt_p.ap()[wrow:wrow + E, 512 * kw:512 * (kw + 1)]
                        .rearrange("(c p) f -> p c f", p=128))
                    for fl in range(4):
                        ft = 4 * kw + fl
                        ps = psmm.tile([128, MT], f32, tag="mm")
                        for ec in range(EC):
                            nc.tensor.matmul(
                                ps[:], wt[:, ec, 128 * fl:128 * (fl + 1)],
                                xt[:, ec, :],
                                start=(ec == 0), stop=(ec == EC - 1))
                        nc.vector.tensor_scalar_add(qt[:, ft, :], ps[:],
                                                    bq[:, ft:ft + 1])

                # ---- keep-warm dummies riding out the trigger+gather window ----
                psd0 = psdz.tile([128, 512], f32, tag="dz")
                for dmy in range(64):
                    nc.tensor.matmul(psd0[:], dz[:, 0:128], dz[:],
                                     start=(dmy == 0), stop=(dmy == 63))

                # ---- PE warm-up probe: a tiny DMA that completes right at
                # gather end, then a few matmuls on it to lift the HAM clock
                # gate back to full speed while the real unpack DMAs land ----
                kprobe = consts.tile([128, 128], f8, tag="kprobe", bufs=2)
                nc.sync.dma_start(
                    kprobe[:],
                    kv_kd[0:128 * 128].rearrange("(p m) -> p m", p=128))
                psd = psdz.tile([128, 512], f32, tag="dz")
                for dmy in range(16):
                    nc.tensor.matmul(psd[:], kprobe[:], dz[:],
                                     start=(dmy == 0), stop=(dmy == 15))

                # ---- unpack gathered K: 8 line-rate DMAs, rank r's 256
                # columns land contiguously at sigma positions (2r, 2r+1);
                # 4 half-K tiles so scores start after the first pair ----
                ktfs = [pktf.tile([128, EC, 512], f8, tag=f"ktf{j}",
                                  name=f"ktf{j}_{li}")
                        for j in range(4)]
                for r in range(NCORES):
                    eng = nc.scalar if r % 2 else nc.sync
                    eng.dma_start(
                        ktfs[r // 2][:, :, MT * (r % 2):MT * (r % 2 + 1)],
                        kv_kd[r * KV_K_ELEMS:(r + 1) * KV_K_ELEMS]
                        .rearrange("(c p m) -> p c m", p=128, m=MT))

                # ---- unpack gathered V (2 strided DMAs, after K) ----
                # vf slot 2r = rank r's tile A (global tile r), slot 2r+1 =
                # rank r's tile B (global tile 15-r) -> sigma order.
                if not last:
                    kv2v = kv_vd[:].rearrange("(r x) -> r x", r=NCORES)
                    vfa = pvf.tile([128, 8, E], f8, tag="vfa")
                    vfb = pvf.tile([128, 8, E], f8, tag="vfb")
                    nc.sync.dma_start(
                        vfa[:],
                        kv2v[:, 0:128 * E]
                        .rearrange("r (p e) -> p r e", p=128))
                    nc.scalar.dma_start(
                        vfb[:],
                        kv2v[:, 128 * E:KV_V_ELEMS]
                        .rearrange("r (p e) -> p r e", p=128))

                # ---- scores + softmax + accumulate, per m-tile ----
                for mt, width, mask_t, acc_t, stag in (
                    (1, 2048, maskb, acc_b, "b"),
                    (0, 1024, maska, acc_a, "a"),
                ):
                    scores = pscore.tile([128, width], f32, tag=f"sc{stag}")
                    expv = pscore.tile([128, width], bf16, tag=f"ex{stag}")
                    rsp = pstats.tile([128, 4], f32, tag="rsp")
                    for ns in range(width // 512):
                        ps = pssc.tile([128, 512], f32, tag="sc")
                        if mt == 0:
                            # tile A attends only to global tiles 0..7 = the
                            # A-half (even sigma) blocks, read strided
                            for h in range(2):
                                ktf_h = ktfs[2 * ns + h]
                                for ec in range(EC):
                                    rhs = (ktf_h[:, ec, :]
                                           .rearrange("p (s m) -> p s m", m=128)
                                           [:, 0:4:2, :])
                                    nc.tensor.matmul(
                                        ps[:, 256 * h:256 * (h + 1)],
                                        qt[:, ec, 0:128], rhs,
                                        start=(ec == 0), stop=(ec == EC - 1))
                        else:
                            for ec in range(EC):
                                nc.tensor.matmul(
                                    ps[:], qt[:, ec, 128:256],
                                    ktfs[ns][:, ec, :],
                                    start=(ec == 0), stop=(ec == EC - 1))
                        nc.vector.scalar_tensor_tensor(
                            out=scores[:, 512 * ns:512 * (ns + 1)],
                            in0=ps[:], scalar=1.0,
                            in1=mask_t[:, 512 * ns:512 * (ns + 1)],
                            op0=AOP.mult, op1=AOP.add)
                        nc.scalar.activation(
                            expv[:, 512 * ns:512 * (ns + 1)],
                            scores[:, 512 * ns:512 * (ns + 1)], AF.Exp,
                            accum_out=rsp[:, ns:ns + 1])
                    rowsum = pstats.tile([128, 1], f32, tag="rs")
                    nc.vector.reduce_sum(out=rowsum[:], in_=rsp[:, 0:width // 512],
                                         axis=mybir.AxisListType.X)
                    recip = pstats.tile([128, 1], f32, tag="rc")
                    nc.vector.reciprocal(recip[:], rowsum[:])
                    if not last:
                        # normalized w, scattered into the interleaved
                        # buffer (emitted before the acc update so the
                        # transposes aren't queued behind it on DVE)
                        ex3 = expv[:].rearrange("p (s m) -> p s m", m=128)
                        if mt == 0:
                            # A position j (global tile j) -> slot 2j
                            nc.vector.tensor_scalar_mul(
                                wab3[:, 0:16:2, :], ex3, recip[:])
                        else:
                            # B position s=2t (tile t) -> slot 2t+1 = s+1
                            nc.vector.tensor_scalar_mul(
                                wab3[:, 1:16:2, :], ex3[:, 0:16:2, :], recip[:])
                            # B position s=2u+1 (tile 15-u) -> slot 31-2u
                            nc.vector.tensor_scalar_mul(
                                wab3[:, 31:16:-2, :], ex3[:, 1:16:2, :], recip[:])
                    # acc += expv * recip (fused; normalized w in f32 never
                    # needs to materialize)
                    nc.vector.scalar_tensor_tensor(
                        out=acc_t[:], in0=expv[:], scalar=recip[:],
                        in1=acc_t[:], op0=AOP.mult, op1=AOP.add)

                if last:
                    continue

                # ---- two batched W^T transposes (off the PE); splitting
                # lets W@V's first half start while the second transposes ----
                wtr1 = pw.tile([128, 16, 128], bf16, tag="wt1")
                wtr2 = pw.tile([128, 16, 128], bf16, tag="wt2")
                nc.sync.dma_start_transpose(wtr1[:], w_ab[:, 0:2048])
                nc.scalar.dma_start_transpose(wtr2[:], w_ab[:, 2048:4096])

                # ---- W @ V -> next layer activation (out-proj folded) ----
                xt_next = px.tile([128, EC, MT], bf16, tag="xt")
                for ep2 in range(EC):
                    ps = pssm.tile([128, MT], f32, tag="sm")
                    for t in range(16):
                        sv = SIGMA_INV[t]
                        vf_h = vfa if sv % 2 == 0 else vfb
                        wtr_h, sl = (wtr1, t) if t < 8 else (wtr2, t - 8)
                        nc.tensor.matmul(
                            ps[:], vf_h[:, sv // 2, 128 * ep2:128 * (ep2 + 1)],
                            wtr_h[:, 2 * sl:2 * sl + 2, :],
                            start=(t == 0), stop=(t == 15))
                    nc.scalar.copy(xt_next[:, ep2, :], ps[:])
                xt = xt_next

            # ---- finalize: mean over layers, write output ----
            out_a = pscore.tile([128, 1024], f32, tag="sca")
            nc.scalar.mul(out_a[:], acc_a[:], 1.0 / L)
            nc.sync.dma_start(out_p[0:128, 0:1024], out_a[:])
            out_b = pscore.tile([128, 2048], f32, tag="scb")
            nc.scalar.mul(out_b[:], acc_b[:], 1.0 / L)
            nc.sync.dma_start(out_p[128:256, :], out_b[:])

    nc.compile()
    return nc


def _prep_in_maps(all_mentions, Wqkv, bqkv, Wo, bo):
    all_mentions = np.asarray(all_mentions, np.float32)
    Wqkv = np.asarray(Wqkv, np.float32)
    bqkv = np.asarray(bqkv, np.float32)
    Wo = np.asarray(Wo, np.float32)
    bo = np.asarray(bo, np.float32)

    # Fold each layer's output projection into the next layer's QKV:
    # qkv_i = wv_{i-1} @ (Wqkv_i @ Wo_{i-1})^T + (bqkv_i + Wqkv_i @ bo_{i-1})
    Wp = np.empty_like(Wqkv)
    bp = np.empty_like(bqkv)
    Wp[0] = Wqkv[0]
    bp[0] = bqkv[0]
    for i in range(1, L):
        Wp[i] = Wqkv[i] @ Wo[i - 1]
        bp[i] = bqkv[i] + Wqkv[i] @ bo[i - 1]
    Wp[:, :E, :] *= SCALE   # torch scales Q by head_dim**-0.5
    bp[:, :E] *= SCALE

    wqkvt = np.ascontiguousarray(
        Wp.transpose(0, 2, 1)).reshape(L * E, 3 * E).astype(BF16)
    bqkv_flat = np.ascontiguousarray(bp.reshape(-1), np.float32)

    p = np.arange(128)
    j1 = np.arange(1024)
    # global column index for sigma-ordered B columns
    jb = (128 * np.asarray(SIGMA_G)[:, None] + np.arange(128)[None, :]).reshape(-1)

    in_maps = []
    for c in range(NCORES):
        ta, tb = c, 15 - c
        rows = np.concatenate([np.arange(128 * ta, 128 * (ta + 1)),
                               np.arange(128 * tb, 128 * (tb + 1))])
        xt = np.ascontiguousarray(all_mentions[rows].T).astype(BF16)
        maska = np.where(j1[None, :] <= (128 * ta + p)[:, None],
                         np.float32(0.0), np.float32(NEG)).astype(BF16)
        maskb = np.where(jb[None, :] <= (128 * tb + p)[:, None],
                         np.float32(0.0), np.float32(NEG)).astype(BF16)
        in_maps.append({
            "xt": xt,
            "wqkvt": wqkvt,
            "bqkv": bqkv_flat,
            "maska": maska,
            "maskb": maskb,
        })
    return in_maps


class Runner:
    def __init__(self):
        self.nc = _build_nc()

    def run(self, in_maps, **kw):
        from concourse.bass_utils import run_bass_kernel_spmd
        return run_bass_kernel_spmd(self.nc, in_maps,
                                    core_ids=list(range(NCORES)), **kw)


def get_runner():
    global _RUNNER
    if _RUNNER is None:
        _RUNNER = Runner()
    return _RUNNER


def assemble_output(results):
    out = np.zeros((N, N), np.float32)
    inv = np.asarray(SIGMA_INV)
    for c in range(NCORES):
        o = np.asarray(results[c]["out"], np.float32)
        out[128 * c:128 * (c + 1), :1024] = o[0:128, :1024]
        ob = o[128:256].reshape(128, 16, 128)
        out[128 * (15 - c):128 * (16 - c), :] = ob[:, inv, :].reshape(128, N)
    return out


def kernel(all_mentions, Wqkv, bqkv, Wo, bo):
    runner = get_runner()
    in_maps = _prep_in_maps(all_mentions, Wqkv, bqkv, Wo, bo)
    res = runner.run(in_maps)
    return assemble_output(res.results)


# revision 38
# speedup vs baseline: 1.0223x; 1.0223x over previous
"""Distributed Trainium2 kernel for the 4-layer single-head causal-attention
stack (returns mean attention weights over layers).

Sharding: sequence-parallel over the 2048 mentions. 16 row-tiles of 128;
core c owns tiles {c, 15-c} so causal-attention work is identical on every
core -> one uniform SPMD program. Per layer each core projects K,V for its
256 rows, all-gathers K,V across the 8 cores (one collective), projects Q
while the gather runs, then computes masked scores, softmax and W@V in
bf16 with f32 PSUM accumulation.

Key structural choices:
- The per-layer output projection is folded into the next layer's QKV
  weights on the host (W'_i = Wqkv_i @ Wo_{i-1}); x_i never materializes.
- Layer 3 only computes Q,K (its attention output is never consumed).
- K/V columns live in "rank-paired" order sigma = [0,15,1,14,...]: rank
  r's gathered block lands contiguously, so the K unpack is 8 line-rate
  DMAs and V unpack is 2 strided DMAs. Row-tile A (global tile c) only
  ever attends to global tiles 0..7 = the even sigma positions, read as a
  strided matmul operand, so the causal 25% FLOP saving survives the
  reordering. The host un-permutes the B rows' output columns.
- W^T for W@V comes from one batched DMA transpose of an interleaved
  [A0 B0 A1 B1 ...] buffer -> N=256 moving operands, no PE transposes.
- DMA count is kept low and split across both HWDGE rings (sync+scalar);
  PSUM->SBUF copies are spread over DVE and ACT.
"""

import numpy as np
import ml_dtypes

N, E, L, NCORES = 2048, 1024, 4, 8
EC = E // 128          # 8 contraction chunks of 128
MT = 256               # mention rows per core
SCALE = 1.0 / np.sqrt(np.float32(E))
KV_K_ELEMS = E * MT            # k block: [1024, 256] (feature-major)
KV_V_ELEMS = MT * E            # v block: [256, 1024] (row-major natural)
KV_ELEMS = KV_K_ELEMS + KV_V_ELEMS
NEG = -1e30

BF16 = ml_dtypes.bfloat16

# sigma: column-block position s holds global row-tile SIGMA_G[s]
SIGMA_G = [t for pair in ((u, 15 - u) for u in range(8)) for t in pair]
# inverse: global tile t lives at column-block position SIGMA_INV[t]
SIGMA_INV = [0] * 16
for _s, _g in enumerate(SIGMA_G):
    SIGMA_INV[_g] = _s

_RUNNER = None


def _build_nc():
    import concourse.mybir as mybir
    import concourse.tile as tile
    from concourse import bacc
    from contextlib import ExitStack

    f32 = mybir.dt.float32
    bf16 = mybir.dt.bfloat16
    f8 = mybir.dt.float8e4

    nc = bacc.Bacc("TRN2", target_bir_lowering=False, debug=False,
                   num_devices=NCORES)

    xt_p = nc.declare_dram_parameter("xt", [E, MT], bf16, isOutput=False)
    wqkvt_p = nc.declare_dram_parameter("wqkvt", [L * E, 3 * E], bf16, isOutput=False)
    bqkv_p = nc.declare_dram_parameter("bqkv", [L * 3 * E], f32, isOutput=False)
    maska_p = nc.declare_dram_parameter("maska", [128, 1024], bf16, isOutput=False)
    maskb_p = nc.declare_dram_parameter("maskb", [128, 2048], bf16, isOutput=False)
    out_p = nc.declare_dram_parameter("out", [MT, N], f32, isOutput=True)

    AOP = mybir.AluOpType
    AF = mybir.ActivationFunctionType

    with tile.TileContext(nc) as tc:
        with ExitStack() as stack:
            ep_ = lambda **kw: stack.enter_context(tc.tile_pool(**kw))
            dram = ep_(name="dram", bufs=2, space="DRAM")
            consts = ep_(name="consts", bufs=1)
            px = ep_(name="px", bufs=2)
            pq = ep_(name="pq", bufs=2)
            pktf = ep_(name="pktf", bufs=1)
            pvf = ep_(name="pvf", bufs=1)
            pscore = ep_(name="pscore", bufs=1)
            pw = ep_(name="pw", bufs=1)
            pacc = ep_(name="pacc", bufs=1)
            pwqk = ep_(name="pwqk", bufs=2)
            pwv = ep_(name="pwv", bufs=2)
            pstage = ep_(name="pstage", bufs=2)
            pbias = ep_(name="pbias", bufs=2)
            pstats = ep_(name="pstats", bufs=4)
            psmm = ep_(name="psmm", bufs=2, space="PSUM")
            pssc = ep_(name="pssc", bufs=3, space="PSUM")
            pssm = ep_(name="pssm", bufs=2, space="PSUM")
            psdz = ep_(name="psdz", bufs=1, space="PSUM")

            maska = consts.tile([128, 1024], bf16)
            nc.sync.dma_start(maska[:], maska_p[:, :])
            maskb = consts.tile([128, 2048], bf16)
            nc.sync.dma_start(maskb[:], maskb_p[:, :])
            zeros = consts.tile([128, 1024], f32)
            nc.vector.memset(zeros[:], 0.0)
            nc.sync.dma_start(out_p[0:128, 1024:2048], zeros[:])
            acc_a = pacc.tile([128, 1024], f32, tag="acca")
            nc.vector.memset(acc_a[:], 0.0)
            acc_b = pacc.tile([128, 2048], f32, tag="accb")
            nc.vector.memset(acc_b[:], 0.0)

            # zero scratch for PE keep-warm dummy matmuls (the HAM clock
            # gate halves the PE clock after ~4us idle; gathers idle the PE
            # for ~40-55us, so every post-gather matmul would run cold)
            dz = consts.tile([128, 512], bf16)
            nc.vector.memset(dz[:], 0.0)

            # w_ab slot 2t = tile-A's global-tile-t block (stays zero for
            # t>=8), slot 2t+1 = tile-B's global-tile-t block; reused (and
            # the zero slots memset) once across all layers
            w_ab = pw.tile([128, 4096], bf16, tag="wab")
            wab3 = w_ab[:].rearrange("p (s m) -> p s m", m=128)
            nc.gpsimd.memset(wab3[:, 16:32:2, :], 0.0)

            xt = px.tile([128, EC, MT], bf16, tag="xt")
            nc.sync.dma_start(
                xt[:], xt_p.ap().rearrange("(c p) m -> p c m", p=128))

            for li in range(L):
                last = li == L - 1
                wrow = li * E  # weight row offset for this layer

                bq = pbias.tile([128, 24], f32, tag="bq")
                nc.sync.dma_start(
                    bq[:],
                    bqkv_p.ap()[li * 3 * E:(li + 1) * 3 * E]
                    .rearrange("(c p) -> p c", p=128))

                kv_ks = dram.tile([KV_K_ELEMS], f8, tag="kvks")
                kv_kd = dram.tile([KV_K_ELEMS * NCORES], f8, tag="kvkd",
                                  addr_space="Shared")

                # ---- K projection (features 1024:2048 -> f_tiles 8..15) ----
                kstage = pstage.tile([128, 8, MT], f8, tag="kst")
                for kw in range(2):
                    wt = pwqk.tile([128, EC, 512], bf16, tag="wqk")
                    nc.sync.dma_start(
                        wt[:],
                        wqkvt_p.ap()[wrow:wrow + E,
                                     1024 + 512 * kw:1024 + 512 * (kw + 1)]
                        .rearrange("(c p) f -> p c f", p=128))
                    for fl in range(4):
                        ft = 8 + 4 * kw + fl
                        ps = psmm.tile([128, MT], f32, tag="mm")
                        for ec in range(EC):
                            nc.tensor.matmul(
                                ps[:], wt[:, ec, 128 * fl:128 * (fl + 1)],
                                xt[:, ec, :],
                                start=(ec == 0), stop=(ec == EC - 1))
                        nc.vector.tensor_scalar_add(kstage[:, ft - 8, :], ps[:],
                                                    bq[:, ft:ft + 1])
                nc.sync.dma_start(
                    kv_ks[:].rearrange("(c p m) -> p c m", p=128, m=MT),
                    kstage[:])
                nc.gpsimd.collective_compute(
                    "AllGather", AOP.bypass,
                    replica_groups=[list(range(NCORES))],
                    ins=[kv_ks[:].opt()],
                    outs=[kv_kd[:].opt()],
                )

                # ---- V projection (natural layout [m, e]) ----
                if not last:
                    kv_vs = dram.tile([KV_V_ELEMS], f8, tag="kvvs")
                    kv_vd = dram.tile([KV_V_ELEMS * NCORES], f8, tag="kvvd",
                                      addr_space="Shared")
                    vstage = pstage.tile([128, 2, E], f8, tag="vst")
                    for s in range(2):
                        wvt_w = pwv.tile([128, EC, 512], bf16, tag="wv")
                        nc.sync.dma_start(
                            wvt_w[:],
                            wqkvt_p.ap()[wrow:wrow + E,
                                         2048 + 512 * s:2048 + 512 * (s + 1)]
                            .rearrange("(c p) f -> p c f", p=128))
                        for mt in range(2):
                            ps = psmm.tile([128, 512], f32, tag="mm")
                            for ec in range(EC):
                                nc.tensor.matmul(
                                    ps[:], xt[:, ec, 128 * mt:128 * (mt + 1)],
                                    wvt_w[:, ec, :],
                                    start=(ec == 0), stop=(ec == EC - 1))
                            nc.scalar.copy(vstage[:, mt, 512 * s:512 * (s + 1)],
                                           ps[:])
                    nc.sync.dma_start(
                        kv_vs[:].rearrange("(t p e) -> p t e", t=2, p=128),
                        vstage[:])
                    nc.gpsimd.collective_compute(
                        "AllGather", AOP.bypass,
                        replica_groups=[list(range(NCORES))],
                        ins=[kv_vs[:].opt()],
                        outs=[kv_vd[:].opt()],
                    )

                # ---- Q projection (features 0:1024, pre-scaled weights) ----
                qt = pq.tile([128, EC, MT], bf16, tag="qt")
                for kw in range(2):
                    wt = pwqk.tile([128, EC, 512], bf16, tag="wqk")
                    nc.sync.dma_start(
                        wt[:],
                        wqkvt_p.ap()[wrow:wrow + E, 512 * kw:512 * (kw + 1)]
                        .rearrange("(c p) f -> p c f", p=128))
                    for fl in range(4):
                        ft = 4 * kw + fl
                        ps = psmm.tile([128, MT], f32, tag="mm")
                        for ec in range(EC):
                            nc.tensor.matmul(
                                ps[:], wt[:, ec, 128 * fl:128 * (fl + 1)],
                                xt[:, ec, :],
                                start=(ec == 0), stop=(ec == EC - 1))
                        nc.vector.tensor_scalar_add(qt[:, ft, :], ps[:],
                                                    bq[:, ft:ft + 1])

                # ---- keep-warm dummies riding out the trigger+gather
                # window: an ungated block fills the pre-trigger bubble (and
                # the start barrier on layer 0), then a block gated on a
                # kv_ks readback covers the gather itself ----
                nd0 = 128 if li == 0 else 48
                psd0 = psdz.tile([128, 512], f32, tag="dz")
                for dmy in range(nd0):
                    nc.tensor.matmul(psd0[:], dz[:, 0:128], dz[:],
                                     start=(dmy == 0), stop=(dmy == nd0 - 1))
                kready = consts.tile([128, 128], f8, tag="kready", bufs=2)
                nc.sync.dma_start(
                    kready[:],
                    kv_ks[0:128 * 128].rearrange("(p m) -> p m", p=128))
                psd1 = psdz.tile([128, 512], f32, tag="dz")
                for dmy in range(32):
                    nc.tensor.matmul(psd1[:], kready[:], dz[:],
                                     start=(dmy == 0), stop=(dmy == 31))

                # ---- PE warm-up probe: a tiny DMA that completes right at
                # gather end, then a few matmuls on it to lift the HAM clock
                # gate back to full speed while the real unpack DMAs land ----
                kprobe = consts.tile([128, 128], f8, tag="kprobe", bufs=2)
                nc.sync.dma_start(
                    kprobe[:],
                    kv_kd[0:128 * 128].rearrange("(p m) -> p m", p=128))
                psd = psdz.tile([128, 512], f32, tag="dz")
                for dmy in range(16):
                    nc.tensor.matmul(psd[:], kprobe[:], dz[:],
                                     start=(dmy == 0), stop=(dmy == 15))

                # ---- unpack gathered K: 8 line-rate DMAs, rank r's 256
                # columns land contiguously at sigma positions (2r, 2r+1);
                # 4 half-K tiles so scores start after the first pair ----
                ktfs = [pktf.tile([128, EC, 512], f8, tag=f"ktf{j}",
                                  name=f"ktf{j}_{li}")
                        for j in range(4)]
                for r in range(NCORES):
                    eng = nc.scalar if r % 2 else nc.sync
                    eng.dma_start(
                        ktfs[r // 2][:, :, MT * (r % 2):MT * (r % 2 + 1)],
                        kv_kd[r * KV_K_ELEMS:(r + 1) * KV_K_ELEMS]
                        .rearrange("(c p m) -> p c m", p=128, m=MT))

                # ---- unpack gathered V (2 strided DMAs, after K) ----
                # vf slot 2r = rank r's tile A (global tile r), slot 2r+1 =
                # rank r's tile B (global tile 15-r) -> sigma order.
                if not last:
                    kv2v = kv_vd[:].rearrange("(r x) -> r x", r=NCORES)
                    vfa = pvf.tile([128, 8, E], f8, tag="vfa")
                    vfb = pvf.tile([128, 8, E], f8, tag="vfb")
                    nc.sync.dma_start(
                        vfa[:],
                        kv2v[:, 0:128 * E]
                        .rearrange("r (p e) -> p r e", p=128))
                    nc.scalar.dma_start(
                        vfb[:],
                        kv2v[:, 128 * E:KV_V_ELEMS]
                        .rearrange("r (p e) -> p r e", p=128))

                # ---- scores + softmax + accumulate, per m-tile ----
                for mt, width, mask_t, acc_t, stag in (
                    (1, 2048, maskb, acc_b, "b"),
                    (0, 1024, maska, acc_a, "a"),
                ):
                    scores = pscore.tile([128, width], f32, tag=f"sc{stag}")
                    expv = pscore.tile([128, width], bf16, tag=f"ex{stag}")
                    rsp = pstats.tile([128, 4], f32, tag="rsp")
                    for ns in range(width // 512):
                        ps = pssc.tile([128, 512], f32, tag="sc")
                        if mt == 0:
                            # tile A attends only to global tiles 0..7 = the
                            # A-half (even sigma) blocks, read strided
                            for h in range(2):
                                ktf_h = ktfs[2 * ns + h]
                                for ec in range(EC):
                                    rhs = (ktf_h[:, ec, :]
                                           .rearrange("p (s m) -> p s m", m=128)
                                           [:, 0:4:2, :])
                                    nc.tensor.matmul(
                                        ps[:, 256 * h:256 * (h + 1)],
                                        qt[:, ec, 0:128], rhs,
                                        start=(ec == 0), stop=(ec == EC - 1))
                        else:
                            for ec in range(EC):
                                nc.tensor.matmul(
                                    ps[:], qt[:, ec, 128:256],
                                    ktfs[ns][:, ec, :],
                                    start=(ec == 0), stop=(ec == EC - 1))
                        nc.vector.scalar_tensor_tensor(
                            out=scores[:, 512 * ns:512 * (ns + 1)],
                            in0=ps[:], scalar=1.0,
                            in1=mask_t[:, 512 * ns:512 * (ns + 1)],
                            op0=AOP.mult, op1=AOP.add)
                        nc.scalar.activation(
                            expv[:, 512 * ns:512 * (ns + 1)],
                            scores[:, 512 * ns:512 * (ns + 1)], AF.Exp,
                            accum_out=rsp[:, ns:ns + 1])
                    rowsum = pstats.tile([128, 1], f32, tag="rs")
                    nc.vector.reduce_sum(out=rowsum[:], in_=rsp[:, 0:width // 512],
                                         axis=mybir.AxisListType.X)
                    recip = pstats.tile([128, 1], f32, tag="rc")
                    nc.vector.reciprocal(recip[:], rowsum[:])
                    if not last:
                        # normalized w, scattered into the interleaved
                        # buffer (emitted before the acc update so the
                        # transposes aren't queued behind it on DVE)
                        ex3 = expv[:].rearrange("p (s m) -> p s m", m=128)
                        if mt == 0:
                            # A position j (global tile j) -> slot 2j
                            nc.vector.tensor_scalar_mul(
                                wab3[:, 0:16:2, :], ex3, recip[:])
                        else:
                            # B position s=2t (tile t) -> slot 2t+1 = s+1
                            nc.vector.tensor_scalar_mul(
                                wab3[:, 1:16:2, :], ex3[:, 0:16:2, :], recip[:])
                            # B position s=2u+1 (tile 15-u) -> slot 31-2u
                            nc.vector.tensor_scalar_mul(
                                wab3[:, 31:16:-2, :], ex3[:, 1:16:2, :], recip[:])
                    # acc += expv * recip (fused; normalized w in f32 never
                    # needs to materialize)
                    nc.vector.scalar_tensor_tensor(
                        out=acc_t[:], in0=expv[:], scalar=recip[:],
                        in1=acc_t[:], op0=AOP.mult, op1=AOP.add)

                if last:
                    continue

                # ---- two batched W^T transposes (off the PE); splitting
                # lets W@V's first half start while the second transposes ----
                wtr1 = pw.tile([128, 16, 128], bf16, tag="wt1")
                wtr2 = pw.tile([128, 16, 128], bf16, tag="wt2")
                nc.sync.dma_start_transpose(wtr1[:], w_ab[:, 0:2048])
                nc.scalar.dma_start_transpose(wtr2[:], w_ab[:, 2048:4096])

                # ---- W @ V -> next layer activation (out-proj folded) ----
                xt_next = px.tile([128, EC, MT], bf16, tag="xt")
                for ep2 in range(EC):
                    ps = pssm.tile([128, MT], f32, tag="sm")
                    for t in range(16):
                        sv = SIGMA_INV[t]
                        vf_h = vfa if sv % 2 == 0 else vfb
                        wtr_h, sl = (wtr1, t) if t < 8 else (wtr2, t - 8)
                        nc.tensor.matmul(
                            ps[:], vf_h[:, sv // 2, 128 * ep2:128 * (ep2 + 1)],
                            wtr_h[:, 2 * sl:2 * sl + 2, :],
                            start=(t == 0), stop=(t == 15))
                    nc.scalar.copy(xt_next[:, ep2, :], ps[:])
                xt = xt_next

            # ---- finalize: mean over layers, write output ----
            out_a = pscore.tile([128, 1024], f32, tag="sca")
            nc.scalar.mul(out_a[:], acc_a[:], 1.0 / L)
            nc.sync.dma_start(out_p[0:128, 0:1024], out_a[:])
            out_b = pscore.tile([128, 2048], f32, tag="scb")
            nc.scalar.mul(out_b[:], acc_b[:], 1.0 / L)
            nc.sync.dma_start(out_p[128:256, :], out_b[:])

    nc.compile()
    return nc


def _prep_in_maps(all_mentions, Wqkv, bqkv, Wo, bo):
    all_mentions = np.asarray(all_mentions, np.float32)
    Wqkv = np.asarray(Wqkv, np.float32)
    bqkv = np.asarray(bqkv, np.float32)
    Wo = np.asarray(Wo, np.float32)
    bo = np.asarray(bo, np.float32)

    # Fold each layer's output projection into the next layer's QKV:
    # qkv_i = wv_{i-1} @ (Wqkv_i @ Wo_{i-1})^T + (bqkv_i + Wqkv_i @ bo_{i-1})
    Wp = np.empty_like(Wqkv)
    bp = np.empty_like(bqkv)
    Wp[0] = Wqkv[0]
    bp[0] = bqkv[0]
    for i in range(1, L):
        Wp[i] = Wqkv[i] @ Wo[i - 1]
        bp[i] = bqkv[i] + Wqkv[i] @ bo[i - 1]
    Wp[:, :E, :] *= SCALE   # torch scales Q by head_dim**-0.5
    bp[:, :E] *= SCALE

    wqkvt = np.ascontiguousarray(
        Wp.transpose(0, 2, 1)).reshape(L * E, 3 * E).astype(BF16)
    bqkv_flat = np.ascontiguousarray(bp.reshape(-1), np.float32)

    p = np.arange(128)
    j1 = np.arange(1024)
    # global column index for sigma-ordered B columns
    jb = (128 * np.asarray(SIGMA_G)[:, None] + np.arange(128)[None, :]).reshape(-1)

    in_maps = []
    for c in range(NCORES):
        ta, tb = c, 15 - c
        rows = np.concatenate([np.arange(128 * ta, 128 * (ta + 1)),
                               np.arange(128 * tb, 128 * (tb + 1))])
        xt = np.ascontiguousarray(all_mentions[rows].T).astype(BF16)
        maska = np.where(j1[None, :] <= (128 * ta + p)[:, None],
                         np.float32(0.0), np.float32(NEG)).astype(BF16)
        maskb = np.where(jb[None, :] <= (128 * tb + p)[:, None],
                         np.float32(0.0), np.float32(NEG)).astype(BF16)
        in_maps.append({
            "xt": xt,
            "wqkvt": wqkvt,
            "bqkv": bqkv_flat,
            "maska": maska,
            "maskb": maskb,
        })
    return in_maps


class Runner:
    def __init__(self):
        self.nc = _build_nc()

    def run(self, in_maps, **kw):
        from concourse.bass_utils import run_bass_kernel_spmd
        return run_bass_kernel_spmd(self.nc, in_maps,
                                    core_ids=list(range(NCORES)), **kw)


def get_runner():
    global _RUNNER
    if _RUNNER is None:
        _RUNNER = Runner()
    return _RUNNER


def assemble_output(results):
    out = np.zeros((N, N), np.float32)
    inv = np.asarray(SIGMA_INV)
    for c in range(NCORES):
        o = np.asarray(results[c]["out"], np.float32)
        out[128 * c:128 * (c + 1), :1024] = o[0:128, :1024]
        ob = o[128:256].reshape(128, 16, 128)
        out[128 * (15 - c):128 * (16 - c), :] = ob[:, inv, :].reshape(128, N)
    return out


def kernel(all_mentions, Wqkv, bqkv, Wo, bo):
    runner = get_runner()
    in_maps = _prep_in_maps(all_mentions, Wqkv, bqkv, Wo, bo)
    res = runner.run(in_maps)
    return assemble_output(res.results)
